# revision 2
# baseline (speedup 1.0000x reference)
"""Trainium2 Bass kernel for nn_CLARM_56693568307877 (v2, fp8 DoubleRow).

Computes, for feature sets A [64,640,14,14] and B [128,640,14,14] and a QKV
projection W [240,640]:
    q,k,v = split(x^T W^T); S = q_b k_a^T / sqrt(80); P = softmax(S)
    rec = P v_a;  sim[b,a] = -||v_b - rec||^2_F
Output [128, 64] fp32.

Sharding: data-parallel over the b batch (16 per core x 8 cores);
features_a / W replicated.

v2 design (per core: 16 b x 64 a, N=196 tokens, D=80):
  All matmuls run fp8e4 with DoubleRow perf mode (0.5 cyc/row):
    qkv:   x fp8 [128,5,196], W fp8; q/k produced as two 40-col groups so
           the [40,2,196] (d folded 2x40) DR layout for mm1 falls out of a
           plain 2-bank PSUM->SBUF copy; v_a is produced directly n-major
           (stationary = x chunk, moving = W_v) into vaug [128,2(mc),80].
    mm1:   S^T = k^T.T q^T per (a, 4b): 4 DR matmuls -> 4 PSUM banks.
    exp:   one wide ACT instruction over all 4 banks, bias -2.5 folded
           (keeps E in fp8e4 range), output E fp8 [128,2(mc),784].
    mm2:   U = vaug^T E: 2 DR matmuls (contraction 196 fits one 256-row
           DR pass) -> U [80, 784] PSUM.
  Epilogue avoids any w-broadcast / den-gather DMAs via
    sim = sum_n (2 alpha[n] w[n] - beta[n] w[n]^2) - ||v_b||^2,
    beta = sum_d U^2, alpha = sum_d U v_b, w = 1/(sum_m E):
  P2=U*U and Pv=U*(2 v_b) (DVE/Pool) are reduced over d by one-hot
  stationary PE matmuls that accumulate each pair's row into a wave-shared
  PSUM bank ([64, 392] = beta|alpha per pair); denominators s likewise via
  a ones-stationary DR matmul over E into a second wave bank. The per-wave
  tail is 6 small DVE ops + one output DMA.

Note: this walrus build accepts at most one semaphore wait per instruction
(_split_multi_waits), rejects InstTensorTensorReduce / custom DVE ops /
gpsimd-PSUM access / partition_broadcast.
"""

import numpy as np
import ml_dtypes

import concourse.bass as bass
import concourse.tile as tile
from concourse import mybir
from concourse.bass_utils import run_bass_kernel_spmd

BF16 = mybir.dt.bfloat16
F32 = mybir.dt.float32
FP8 = mybir.dt.float8e4
DR = mybir.MatmulPerfMode.DoubleRow

NCORES = 8
A_FULL = 64
B_FULL = 128
HID = 640
KC = HID // 128  # 5
N = 196          # tokens (14*14)
D = 80           # inner dim
EXP_SHIFT = 2.5  # exp(S - EXP_SHIFT); cancels in softmax, keeps E in fp8 range
SCALE4 = 1.0 / (D ** 0.25)  # folded into both W_q and W_k

_PROGRAM_CACHE = {}


def _build(Asz, Bsz):
    assert Bsz % 4 == 0 and Asz % 4 == 0
    NSG = Bsz // 4              # subgroups per a (4 b's each)
    WA = 8                      # a's per wave
    PW = WA * Bsz               # pairs per wave (128)

    nc = bass.Bass("TRN2", debug=False)
    fa = nc.dram_tensor("fa", [Asz, KC, 128, N], FP8, kind="ExternalInput")
    fb = nc.dram_tensor("fb", [Bsz, KC, 128, N], FP8, kind="ExternalInput")
    wt = nc.dram_tensor("wt", [KC, 128, 240], FP8, kind="ExternalInput")
    simo = nc.dram_tensor("sim", [Asz, Bsz], F32, kind="ExternalOutput")

    Exp = mybir.ActivationFunctionType.Exp
    mult = mybir.AluOpType.mult
    sub = mybir.AluOpType.subtract
    add = mybir.AluOpType.add
    X = mybir.AxisListType.X

    with tile.TileContext(nc) as tc:
        with (
            tc.tile_pool(name="const", bufs=1) as cpool,
            tc.tile_pool(name="ring", bufs=1, space="PSUM") as rpool,
            tc.tile_pool(name="x", bufs=3) as x_pool,
            tc.tile_pool(name="e", bufs=2) as e_pool,
            tc.tile_pool(name="u", bufs=2) as u_pool,
            tc.tile_pool(name="t", bufs=3) as t_pool,
            tc.tile_pool(name="wv", bufs=2) as wv_pool,
        ):
            wt_sb = cpool.tile([128, KC, 240], FP8, tag="wt")
            kT = cpool.tile([40, 2, Asz, N], FP8, tag="kT")
            qT = cpool.tile([40, 2, Bsz, N], FP8, tag="qT")
            vaug = cpool.tile([128, 2, Asz, D], FP8, tag="vaug")
            vbT2 = cpool.tile([D, Bsz, N], BF16, tag="vbT2")
            ohb = cpool.tile([D, 63], BF16, tag="ohb")       # one-hot cols @31
            ohs = cpool.tile([128, 2, 64], FP8, tag="ohs")   # ones col @31
            ebias = cpool.tile([128, 1], F32, tag="ebias")
            vn64 = cpool.tile([PW, 1], F32, tag="vn64")
            ring = rpool.tile([128, 8, 512], F32, tag="ring")

            nc.sync.dma_start(wt_sb, wt.ap().rearrange("k p c -> p k c"))
            nc.gpsimd.memset(vaug[:], 0.0)   # mc1 rows 68:128 must stay 0
            nc.gpsimd.memset(ohb[:], 0.0)
            nc.gpsimd.memset(ohb[:, 31:32], 1.0)
            nc.gpsimd.memset(ohs[:], 0.0)
            nc.gpsimd.memset(ohs[:, 0, 31:32], 1.0)
            nc.gpsimd.memset(ohs[0:68, 1, 31:32], 1.0)  # mc1 valid rows only
            nc.gpsimd.memset(ebias[:], -EXP_SHIFT)
            # HW PSUM powers up with undefined bits; exp reads rows 68:128 of
            # the mc1 mm1 banks (never written by the 68-row matmuls), so any
            # stale NaN there would poison mm2 via NaN*0. Zero them once.
            nc.vector.memset(ring[64:128, 0:8, :], 0.0)

            def qkv_mm(xt, c0, slot, nrows):
                """Accumulate W[:, c0:c0+nrows]^T x into ring slot.
                3 matmuls: 2 DoubleRow over kc pairs + 1 plain for kc 4."""
                out = ring[0:nrows, slot, 0:N]
                for t in range(2):
                    nc.tensor.matmul(
                        out, wt_sb[:, 2 * t:2 * t + 2, c0:c0 + nrows],
                        xt[:, 2 * t:2 * t + 2, 0:N],
                        start=(t == 0), stop=False, perf_mode=DR)
                nc.tensor.matmul(out, wt_sb[:, 4, c0:c0 + nrows],
                                 xt[:, 4, 0:N], start=False, stop=True)

            # ---- phase 1b: q, v for the 16 b batches; ||v_b||^2 ----
            for b in range(Bsz):
                xt = x_pool.tile([128, KC, 224], FP8, tag="x")
                nc.sync.dma_start(xt[:, :, 0:N], fb[b].rearrange("k p n -> p k n"))
                s0 = 0 if b % 2 == 0 else 4
                qkv_mm(xt, 0, s0, 40)     # q cols 0:40  -> d 0:40
                qkv_mm(xt, 40, s0 + 1, 40)  # q cols 40:80 -> d 40:80
                qkv_mm(xt, 160, s0 + 2, D)  # v (d-major, 80 rows)
                if b % 2 == 0:
                    nc.vector.tensor_copy(qT[:, :, b, :], ring[0:40, s0:s0 + 2, 0:N])
                else:
                    nc.scalar.copy(qT[:, :, b, :], ring[0:40, s0:s0 + 2, 0:N])
                nc.scalar.mul(vbT2[:, b, :], ring[0:D, s0 + 2, 0:N], 2.0)
                # ||2 v_b||^2 per token -> hot row b of slot 3
                vsq = t_pool.tile([D, N], BF16, tag="vsq")
                nc.vector.tensor_tensor(vsq, vbT2[:, b, :], vbT2[:, b, :], op=mult)
                nc.tensor.matmul(ring[0:32, 3, 0:N], ohb[:, 31 - b:63 - b], vsq,
                                 start=(b == 0), stop=(b == Bsz - 1),
                                 tile_position=(0, 0))
            vn16 = cpool.tile([Bsz, 1], F32, tag="vn16")
            nc.vector.reduce_sum(out=vn16, in_=ring[0:Bsz, 3, 0:N], axis=X)
            for g in range(PW // Bsz):
                nc.sync.dma_start(vn64[g * Bsz:(g + 1) * Bsz, :], vn16[:])

            # ---- phase 1a: k, vaug for the 64 a batches ----
            for a in range(Asz):
                xt = x_pool.tile([128, KC, 224], FP8, tag="x")
                nc.sync.dma_start(xt[:, :, 0:N], fa[a].rearrange("k p n -> p k n"))
                s0 = 0 if a % 2 == 0 else 4
                qkv_mm(xt, 80, s0, 40)    # k cols 80:120  -> d 0:40
                qkv_mm(xt, 120, s0 + 1, 40)  # k cols 120:160 -> d 40:80
                # v_a n-major: stationary = x chunk, moving = W_v
                for mc in range(2):
                    nrow = 128 if mc == 0 else 68
                    out = ring[0:nrow, s0 + 2 + mc, 0:D]
                    for t in range(2):
                        nc.tensor.matmul(
                            out, xt[:, 2 * t:2 * t + 2, mc * 128:mc * 128 + nrow],
                            wt_sb[:, 2 * t:2 * t + 2, 160:240],
                            start=(t == 0), stop=False, perf_mode=DR)
                    nc.tensor.matmul(out, xt[:, 4, mc * 128:mc * 128 + nrow],
                                     wt_sb[:, 4, 160:240], start=False, stop=True)
                if a % 2 == 0:
                    nc.vector.tensor_copy(kT[:, :, a, :], ring[0:40, s0:s0 + 2, 0:N])
                    nc.scalar.copy(vaug[0:128, 0, a, :], ring[0:128, s0 + 2, 0:D])
                    nc.scalar.copy(vaug[0:68, 1, a, :], ring[0:68, s0 + 3, 0:D])
                else:
                    nc.scalar.copy(kT[:, :, a, :], ring[0:40, s0:s0 + 2, 0:N])
                    nc.vector.tensor_copy(vaug[0:128, 0, a, :], ring[0:128, s0 + 2, 0:D])
                    nc.vector.tensor_copy(vaug[0:68, 1, a, :], ring[0:68, s0 + 3, 0:D])

            # ---- phase 2: 16 waves x (4 a x 4 subgroups) ----
            # 2-deep software pipeline per wave:
            #   sg t emits: mm1(t), exp(t) | mm2/s/egress/P2/Pv of t-1 |
            #   alpha/beta matmuls of t-2 (so they never head-block mm1).

            def emit_early(p):
                es, a, jbase = p
                # mm2: U = vaug^T E  [80, 784] into slots 4,5
                for ncx in range(2):
                    nc.tensor.matmul(
                        ring[0:D, 4 + ncx, 0:392], vaug[:, :, a, :],
                        es[:, :, ncx * 392:(ncx + 1) * 392],
                        start=True, stop=True, perf_mode=DR)
                # denominators s -> hot rows of slot 7 (DR only legal at
                # col base 0; other blocks use plain fp8 with mc accumulate)
                for p_ in range(4):
                    j = jbase + p_
                    blk, off = divmod(j, 32)
                    out = ring[32 * blk:32 * blk + 32, 7, 0:N]
                    if blk == 0:
                        nc.tensor.matmul(
                            out, ohs[:, :, 31 - off:63 - off],
                            es[:, :, p_ * N:(p_ + 1) * N],
                            start=(off == 0), stop=(off == 31),
                            perf_mode=DR, tile_position=(0, 0))
                    else:
                        for mc in range(2):
                            nc.tensor.matmul(
                                out, ohs[:, mc, 31 - off:63 - off],
                                es[:, mc, p_ * N:(p_ + 1) * N],
                                start=(off == 0 and mc == 0),
                                stop=(off == 31 and mc == 1),
                                tile_position=(0, 32 * blk))
                # U egress + P2/Pv products
                u_sb = u_pool.tile([D, 2, 392], BF16, tag="u")
                nc.vector.tensor_copy(u_sb, ring[0:D, 4:6, 0:392])
                tt = t_pool.tile([D, 4, 2, N], BF16, tag="t")
                uv = u_sb.rearrange("p c x -> p (c x)").rearrange(
                    "p (j n) -> p j n", j=4)
                nc.vector.tensor_tensor(tt[:, :, 0, :], uv, uv, op=mult)
                nc.gpsimd.tensor_tensor(
                    tt[:, :, 1, :], uv,
                    vbT2[:, jbase % Bsz:jbase % Bsz + 4, :], op=mult)
                return tt, jbase

            def emit_late(p):
                tt, jbase = p
                # beta|alpha -> hot rows of slot 6
                for p_ in range(4):
                    j = jbase + p_
                    blk, off = divmod(j, 32)
                    nc.tensor.matmul(
                        ring[32 * blk:32 * blk + 32, 6, 0:392],
                        ohb[:, 31 - off:63 - off], tt[:, p_, :, :],
                        start=(off == 0), stop=(off == 31),
                        tile_position=(0, 32 * blk))

            for w in range(Asz // WA):
                early_q = None
                late_q = []
                for a_loc in range(WA):
                    a = WA * w + a_loc
                    for sgb in range(NSG):
                        b0 = 4 * sgb
                        jbase = Bsz * a_loc + b0
                        es = e_pool.tile([128, 2, 784], FP8, tag="e")
                        # mm1 mc0 -> slots 0,1; exp_a unblocks as soon as
                        # they land, while PE continues with mc1.
                        for mc in range(2):
                            nrow = 128 if mc == 0 else 68
                            for ncx in range(2):
                                nc.tensor.matmul(
                                    ring[0:nrow, 2 * mc + ncx, 0:392],
                                    kT[:, :, a, mc * 128:mc * 128 + nrow],
                                    qT[:, :, b0 + 2 * ncx:b0 + 2 * ncx + 2, :],
                                    start=True, stop=True, perf_mode=DR)
                            nc.scalar.activation(
                                es[:, mc, :].rearrange(
                                    "p (s y) -> p s y", s=2),
                                ring[:, 2 * mc:2 * mc + 2, 0:392],
                                Exp, bias=ebias[:])
                        if early_q is not None:
                            late_q.append(emit_early(early_q))
                        if len(late_q) == 2:
                            emit_late(late_q.pop(0))
                        early_q = (es, a, jbase)
                late_q.append(emit_early(early_q))
                for p in late_q:
                    emit_late(p)
                # ---- wave epilogue ----
                wb = wv_pool.tile([PW, N], BF16, tag="wb")
                with nc.allow_low_precision(reason="w in bf16 is plenty"):
                    nc.vector.reciprocal(wb, ring[0:PW, 7, 0:N])
                tb = wv_pool.tile([PW, N], BF16, tag="tb")
                nc.vector.tensor_tensor(tb, ring[0:PW, 6, 0:N], wb, op=mult)
                z1 = wv_pool.tile([PW, N], BF16, tag="z1")
                nc.vector.tensor_tensor(
                    z1, ring[0:PW, 6, N:2 * N], tb, op=sub)
                z = wv_pool.tile([PW, N], BF16, tag="z")
                nc.vector.tensor_tensor(z, z1, wb, op=mult)
                rsum = wv_pool.tile([PW, 1], F32, tag="rs")
                nc.vector.reduce_sum(out=rsum, in_=z, axis=X)
                sim64 = wv_pool.tile([PW, 1], F32, tag="sim")
                nc.vector.scalar_tensor_tensor(
                    out=sim64, in0=vn64, scalar=-0.25, in1=rsum,
                    op0=mult, op1=add)
                nc.sync.dma_start(simo[WA * w:WA * (w + 1), :], sim64[:])

    return nc


def _split_multi_waits(nc):
    """This walrus build accepts at most one semaphore wait per instruction;
    Tile emits several (incl. its tail drain). Hoist extra waits onto
    single-wait engine NoOps inserted just before the instruction."""
    cnt = 0
    for f in nc.m.functions:
        for bb in f.blocks:
            insts = list(bb.instructions)
            out = []
            changed = False
            for inst in insts:
                si = getattr(inst, "sync_info", None)
                ws = list(si.on_wait) if (si is not None and si.on_wait) else []
                if len(ws) > 1:
                    changed = True
                    for w in ws[:-1]:
                        cnt += 1
                        out.append(mybir.InstNoOp(
                            name=f"WSPLIT-{cnt}",
                            engine=inst.engine,
                            ins=[], outs=[],
                            sync_info=mybir.SyncInfo(on_wait=[w], on_update=[]),
                        ))
                    si.on_wait = [ws[-1]]
                    inst.sync_info = si
                out.append(inst)
            if changed:
                bb.instructions = out
    return nc


def _get_program(Asz, Bsz):
    key = (Asz, Bsz)
    if key not in _PROGRAM_CACHE:
        _PROGRAM_CACHE[key] = _split_multi_waits(_build(Asz, Bsz))
    return _PROGRAM_CACHE[key]


def _prep_inputs(features_a, features_b, W_qkv, Asz, Bsz, ncores):
    """Host-side: cast to fp8, fold 80^-1/4 into W_q and W_k, reshape."""
    f8 = ml_dtypes.float8_e4m3
    fa = features_a.reshape(Asz, KC, 128, N).astype(f8)
    wt = W_qkv.T.copy().astype(np.float32)   # [640, 240]
    wt[:, 0:2 * D] *= SCALE4
    wt = wt.astype(f8).reshape(KC, 128, 240)
    fbs = []
    for c in range(ncores):
        fbs.append(features_b[c * Bsz:(c + 1) * Bsz]
                   .reshape(Bsz, KC, 128, N).astype(f8))
    return fa, fbs, wt


def kernel(features_a, features_b, W_qkv):
    Asz = features_a.shape[0]
    Bfull = features_b.shape[0]
    ncores = NCORES
    Bsz = Bfull // ncores
    fa, fbs, wt = _prep_inputs(
        np.asarray(features_a), np.asarray(features_b), np.asarray(W_qkv),
        Asz, Bsz, ncores,
    )
    nc = _get_program(Asz, Bsz)
    in_maps = [{"fa": fa, "fb": fbs[c], "wt": wt} for c in range(ncores)]
    res = run_bass_kernel_spmd(nc, in_maps, core_ids=list(range(ncores)))
    out = np.concatenate([res.results[c]["sim"].T for c in range(ncores)], axis=0)
    return out.astype(np.float32)


# revision 3
# speedup vs baseline: 1.0123x; 1.0123x over previous
"""Trainium2 Bass kernel for nn_CLARM_56693568307877 (v2, fp8 DoubleRow).

Computes, for feature sets A [64,640,14,14] and B [128,640,14,14] and a QKV
projection W [240,640]:
    q,k,v = split(x^T W^T); S = q_b k_a^T / sqrt(80); P = softmax(S)
    rec = P v_a;  sim[b,a] = -||v_b - rec||^2_F
Output [128, 64] fp32.

Sharding: data-parallel over the b batch (16 per core x 8 cores);
features_a / W replicated.

v2 design (per core: 16 b x 64 a, N=196 tokens, D=80):
  All matmuls run fp8e4 with DoubleRow perf mode (0.5 cyc/row):
    qkv:   x fp8 [128,5,196], W fp8; q/k produced as two 40-col groups so
           the [40,2,196] (d folded 2x40) DR layout for mm1 falls out of a
           plain 2-bank PSUM->SBUF copy; v_a is produced directly n-major
           (stationary = x chunk, moving = W_v) into vaug [128,2(mc),80].
    mm1:   S^T = k^T.T q^T per (a, 4b): 4 DR matmuls -> 4 PSUM banks.
    exp:   one wide ACT instruction over all 4 banks, bias -2.5 folded
           (keeps E in fp8e4 range), output E fp8 [128,2(mc),784].
    mm2:   U = vaug^T E: 2 DR matmuls (contraction 196 fits one 256-row
           DR pass) -> U [80, 784] PSUM.
  Epilogue avoids any w-broadcast / den-gather DMAs via
    sim = sum_n (2 alpha[n] w[n] - beta[n] w[n]^2) - ||v_b||^2,
    beta = sum_d U^2, alpha = sum_d U v_b, w = 1/(sum_m E):
  P2=U*U and Pv=U*(2 v_b) (DVE/Pool) are reduced over d by one-hot
  stationary PE matmuls that accumulate each pair's row into a wave-shared
  PSUM bank ([64, 392] = beta|alpha per pair); denominators s likewise via
  a ones-stationary DR matmul over E into a second wave bank. The per-wave
  tail is 6 small DVE ops + one output DMA.

Note: this walrus build accepts at most one semaphore wait per instruction
(_split_multi_waits), rejects InstTensorTensorReduce / custom DVE ops /
gpsimd-PSUM access / partition_broadcast.
"""

import numpy as np
import ml_dtypes

import concourse.bass as bass
import concourse.tile as tile
from concourse import mybir
from concourse.bass_utils import run_bass_kernel_spmd

BF16 = mybir.dt.bfloat16
F32 = mybir.dt.float32
FP8 = mybir.dt.float8e4
DR = mybir.MatmulPerfMode.DoubleRow

NCORES = 8
A_FULL = 64
B_FULL = 128
HID = 640
KC = HID // 128  # 5
N = 196          # tokens (14*14)
D = 80           # inner dim
EXP_SHIFT = 2.5  # exp(S - EXP_SHIFT); cancels in softmax, keeps E in fp8 range
SCALE4 = 1.0 / (D ** 0.25)  # folded into both W_q and W_k

_PROGRAM_CACHE = {}


def _build(Asz, Bsz):
    assert Bsz % 4 == 0 and Asz % 4 == 0
    NSG = Bsz // 4              # subgroups per a (4 b's each)
    WA = 8                      # a's per wave
    PW = WA * Bsz               # pairs per wave (128)

    nc = bass.Bass("TRN2", debug=False)
    fa = nc.dram_tensor("fa", [Asz, KC, 128, N], FP8, kind="ExternalInput")
    fb = nc.dram_tensor("fb", [Bsz, KC, 128, N], FP8, kind="ExternalInput")
    wt = nc.dram_tensor("wt", [KC, 128, 240], FP8, kind="ExternalInput")
    simo = nc.dram_tensor("sim", [Asz, Bsz], F32, kind="ExternalOutput")

    Exp = mybir.ActivationFunctionType.Exp
    mult = mybir.AluOpType.mult
    sub = mybir.AluOpType.subtract
    add = mybir.AluOpType.add
    X = mybir.AxisListType.X

    with tile.TileContext(nc) as tc:
        with (
            tc.tile_pool(name="const", bufs=1) as cpool,
            tc.tile_pool(name="ring", bufs=1, space="PSUM") as rpool,
            tc.tile_pool(name="x", bufs=8) as x_pool,
            tc.tile_pool(name="e", bufs=6) as e_pool,
            tc.tile_pool(name="u", bufs=7) as u_pool,
            tc.tile_pool(name="t", bufs=7) as t_pool,
            tc.tile_pool(name="wv", bufs=2) as wv_pool,
        ):
            wt_sb = cpool.tile([128, KC, 240], FP8, tag="wt")
            kT = cpool.tile([40, 2, Asz, N], FP8, tag="kT")
            qT = cpool.tile([40, 2, Bsz, N], FP8, tag="qT")
            vaug = cpool.tile([128, 2, Asz, 97], FP8, tag="vaug")
            vbT2 = cpool.tile([D, Bsz, N], BF16, tag="vbT2")
            ohb = cpool.tile([D, 63], BF16, tag="ohb")       # one-hot cols @31
            ohs96 = cpool.tile([128, 63], BF16, tag="ohs96")  # row 96 hot @31
            ebias = cpool.tile([128, 1], F32, tag="ebias")
            vn64 = cpool.tile([PW, 1], F32, tag="vn64")
            ring = rpool.tile([128, 8, 512], F32, tag="ring")

            nc.sync.dma_start(wt_sb, wt.ap().rearrange("k p c -> p k c"))
            nc.gpsimd.memset(vaug[:], 0.0)   # mc1 rows 68:128 must stay 0
            nc.gpsimd.memset(ohb[:], 0.0)
            nc.gpsimd.memset(ohb[:, 31:32], 1.0)
            nc.gpsimd.memset(ohs96[:], 0.0)
            nc.gpsimd.memset(ohs96[96:97, 31:32], 1.0)
            # ones column at partition 96 of vaug: mm2 then emits the softmax
            # denominator s = sum_m E as row 96 of U for free (mc1 masked to
            # its 68 valid rows)
            nc.gpsimd.memset(vaug[:, 0, :, 96:97], 1.0)
            nc.gpsimd.memset(vaug[0:68, 1, :, 96:97], 1.0)
            nc.gpsimd.memset(ebias[:], -EXP_SHIFT)
            # HW PSUM powers up with undefined bits; exp reads rows 68:128 of
            # the mc1 mm1 banks (never written by the 68-row matmuls), so any
            # stale NaN there would poison mm2 via NaN*0. Zero them once.
            nc.vector.memset(ring[64:128, 0:8, :], 0.0)

            def qkv_mm(xt, c0, slot, nrows):
                """Accumulate W[:, c0:c0+nrows]^T x into ring slot.
                3 matmuls: 2 DoubleRow over kc pairs + 1 plain for kc 4."""
                out = ring[0:nrows, slot, 0:N]
                for t in range(2):
                    nc.tensor.matmul(
                        out, wt_sb[:, 2 * t:2 * t + 2, c0:c0 + nrows],
                        xt[:, 2 * t:2 * t + 2, 0:N],
                        start=(t == 0), stop=False, perf_mode=DR)
                nc.tensor.matmul(out, wt_sb[:, 4, c0:c0 + nrows],
                                 xt[:, 4, 0:N], start=False, stop=True)

            # ---- phase 1b: q, v for the 16 b batches; ||v_b||^2 ----
            for b in range(Bsz):
                xt = x_pool.tile([128, KC, 224], FP8, tag="x")
                nc.sync.dma_start(xt[:, :, 0:N], fb[b].rearrange("k p n -> p k n"))
                s0 = 0 if b % 2 == 0 else 4
                qkv_mm(xt, 0, s0, 40)     # q cols 0:40  -> d 0:40
                qkv_mm(xt, 40, s0 + 1, 40)  # q cols 40:80 -> d 40:80
                qkv_mm(xt, 160, s0 + 2, D)  # v (d-major, 80 rows)
                if b % 2 == 0:
                    nc.vector.tensor_copy(qT[:, :, b, :], ring[0:40, s0:s0 + 2, 0:N])
                else:
                    nc.scalar.copy(qT[:, :, b, :], ring[0:40, s0:s0 + 2, 0:N])
                nc.scalar.mul(vbT2[:, b, :], ring[0:D, s0 + 2, 0:N], 2.0)
                # ||2 v_b||^2 per token -> hot row b of slot 3
                vsq = t_pool.tile([D, N], BF16, tag="vsq")
                nc.vector.tensor_tensor(vsq, vbT2[:, b, :], vbT2[:, b, :], op=mult)
                nc.tensor.matmul(ring[0:32, 3, 0:N], ohb[:, 31 - b:63 - b], vsq,
                                 start=(b == 0), stop=(b == Bsz - 1),
                                 tile_position=(0, 0))
            vn16 = cpool.tile([Bsz, 1], F32, tag="vn16")
            nc.vector.reduce_sum(out=vn16, in_=ring[0:Bsz, 3, 0:N], axis=X)
            for g in range(PW // Bsz):
                nc.sync.dma_start(vn64[g * Bsz:(g + 1) * Bsz, :], vn16[:])

            # ---- phase 1a: k, vaug for the 64 a batches ----
            for a in range(Asz):
                xt = x_pool.tile([128, KC, 224], FP8, tag="x")
                nc.sync.dma_start(xt[:, :, 0:N], fa[a].rearrange("k p n -> p k n"))
                s0 = 0 if a % 2 == 0 else 4
                qkv_mm(xt, 80, s0, 40)    # k cols 80:120  -> d 0:40
                qkv_mm(xt, 120, s0 + 1, 40)  # k cols 120:160 -> d 40:80
                # v_a n-major: stationary = x chunk, moving = W_v
                for mc in range(2):
                    nrow = 128 if mc == 0 else 68
                    out = ring[0:nrow, s0 + 2 + mc, 0:D]
                    for t in range(2):
                        nc.tensor.matmul(
                            out, xt[:, 2 * t:2 * t + 2, mc * 128:mc * 128 + nrow],
                            wt_sb[:, 2 * t:2 * t + 2, 160:240],
                            start=(t == 0), stop=False, perf_mode=DR)
                    nc.tensor.matmul(out, xt[:, 4, mc * 128:mc * 128 + nrow],
                                     wt_sb[:, 4, 160:240], start=False, stop=True)
                if a % 2 == 0:
                    nc.vector.tensor_copy(kT[:, :, a, :], ring[0:40, s0:s0 + 2, 0:N])
                    nc.scalar.copy(vaug[0:128, 0, a, 0:D], ring[0:128, s0 + 2, 0:D])
                    nc.scalar.copy(vaug[0:68, 1, a, 0:D], ring[0:68, s0 + 3, 0:D])
                else:
                    nc.scalar.copy(kT[:, :, a, :], ring[0:40, s0:s0 + 2, 0:N])
                    nc.vector.tensor_copy(vaug[0:128, 0, a, 0:D], ring[0:128, s0 + 2, 0:D])
                    nc.vector.tensor_copy(vaug[0:68, 1, a, 0:D], ring[0:68, s0 + 3, 0:D])

            # ---- phase 2: 16 waves x (4 a x 4 subgroups) ----
            # 2-deep software pipeline per wave:
            #   sg t emits: mm1(t), exp(t) | mm2/s/egress/P2/Pv of t-1 |
            #   alpha/beta matmuls of t-2 (so they never head-block mm1).

            def emit_early(p):
                es, a, jbase = p
                # mm2: U = vaug^T E  [80, 784] into slots 4,5
                with tc.high_priority(offset=500000):
                    for ncx in range(2):
                        nc.tensor.matmul(
                            ring[0:97, 4 + ncx, 0:392], vaug[:, :, a, :],
                            es[:, :, ncx * 392:(ncx + 1) * 392],
                            start=True, stop=True, perf_mode=DR)
                # U egress (row 96 = denominators) + P2/Pv products, in
                # ncx halves so the Pool Pv can start ~1.4us earlier (the
                # egress->Pv leg sits on the pipeline's pacing loop)
                u_sb = u_pool.tile([97, 2, 392], BF16, tag="u")
                tt = t_pool.tile([D, 4, 2, N], BF16, tag="t")
                b0_ = jbase % Bsz
                for h in range(2):
                    nc.vector.tensor_copy(
                        u_sb[:, h, :], ring[0:97, 4 + h, 0:392])
                    uvh = u_sb[0:D, h, :].rearrange("p (j n) -> p j n", j=2)
                    nc.gpsimd.tensor_tensor(
                        tt[:, 2 * h:2 * h + 2, 1, :], uvh,
                        vbT2[:, b0_ + 2 * h:b0_ + 2 * h + 2, :], op=mult)
                    nc.vector.tensor_tensor(
                        tt[:, 2 * h:2 * h + 2, 0, :], uvh, uvh, op=mult)
                return tt, u_sb, jbase

            def emit_late_s(p):
                # denominators s -> hot rows of slot 7 (1-partition
                # stationary reading u_sb row 96)
                tt, u_sb, jbase = p
                for p_ in range(4):
                    j = jbase + p_
                    blk, off = divmod(j, 32)
                    nc.tensor.matmul(
                        ring[32 * blk:32 * blk + 32, 7, 0:N],
                        ohs96[96:97, 31 - off:63 - off],
                        u_sb[96:97, p_ // 2, (p_ % 2) * N:(p_ % 2) * N + N],
                        start=(off == 0), stop=(off == 31),
                        tile_position=(96, 32 * blk))

            def emit_late_ab(p):
                # beta|alpha -> hot rows of slot 6
                tt, u_sb, jbase = p
                for p_ in range(4):
                    j = jbase + p_
                    blk, off = divmod(j, 32)
                    nc.tensor.matmul(
                        ring[32 * blk:32 * blk + 32, 6, 0:392],
                        ohb[:, 31 - off:63 - off], tt[:, p_, :, :],
                        start=(off == 0), stop=(off == 31),
                        tile_position=(0, 32 * blk))

            epi_q = []

            def make_epilogue(w):
                """Per-wave tail as individually emittable thunks so the DVE
                work spreads across the next wave instead of bursting."""
                wb = wv_pool.tile([PW, N], BF16, tag="wb")
                tb = wv_pool.tile([PW, N], BF16, tag="tb")
                z1 = wv_pool.tile([PW, N], BF16, tag="z1")
                z = wv_pool.tile([PW, N], BF16, tag="z")
                rsum = wv_pool.tile([PW, 1], F32, tag="rs")
                sim64 = wv_pool.tile([PW, 1], F32, tag="sim")

                def t0():
                    with nc.allow_low_precision(reason="w in bf16 is plenty"):
                        nc.vector.reciprocal(wb, ring[0:PW, 7, 0:N])
                def t1():
                    nc.vector.tensor_tensor(tb, ring[0:PW, 6, 0:N], wb, op=mult)
                def t2():
                    nc.vector.tensor_tensor(
                        z1, ring[0:PW, 6, N:2 * N], tb, op=sub)
                def t3():
                    nc.vector.tensor_tensor(z, z1, wb, op=mult)
                def t4():
                    nc.vector.reduce_sum(out=rsum, in_=z, axis=X)
                    nc.vector.scalar_tensor_tensor(
                        out=sim64, in0=vn64, scalar=-0.25, in1=rsum,
                        op0=mult, op1=add)
                    nc.sync.dma_start(simo[WA * w:WA * (w + 1), :], sim64[:])
                return [t0, t1, t2, t3, t4]

            for w in range(Asz // WA):
                early_q = None
                late_q = []
                for a_loc in range(WA):
                    a = WA * w + a_loc
                    for sgb in range(NSG):
                        b0 = 4 * sgb
                        jbase = Bsz * a_loc + b0
                        es = e_pool.tile([128, 2, 784], FP8, tag="e")
                        # mm1 mc0 -> slots 0,1; exp_a is gated only by these
                        # two matmuls, so keep everything else off the PE
                        # queue until they are emitted; slip the old s / ab
                        # one-hot matmuls between the two exp halves.
                        pop = late_q.pop(0) if len(late_q) == 5 else None
                        # mm1+exp get top scheduler priority so the list
                        # scheduler never parks late-stage backlog in front
                        # of the exp-critical matmuls.
                        with tc.high_priority(offset=1000000):
                            for mc in range(2):
                                nrow = 128 if mc == 0 else 68
                                for ncx in range(2):
                                    nc.tensor.matmul(
                                        ring[0:nrow, 2 * mc + ncx, 0:392],
                                        kT[:, :, a, mc * 128:mc * 128 + nrow],
                                        qT[:, :, b0 + 2 * ncx:b0 + 2 * ncx + 2, :],
                                        start=True, stop=True, perf_mode=DR)
                                nc.scalar.activation(
                                    es[:, mc, :].rearrange(
                                        "p (s y) -> p s y", s=2),
                                    ring[:, 2 * mc:2 * mc + 2, 0:392],
                                    Exp, bias=ebias[:])
                        if pop is not None:
                            emit_late_s(pop)
                        if early_q is not None:
                            late_q.append(emit_early(early_q))
                        if pop is not None:
                            emit_late_ab(pop)
                        if epi_q:
                            epi_q.pop(0)()
                        early_q = (es, a, jbase)
                late_q.append(emit_early(early_q))
                for p in late_q:
                    emit_late_s(p)
                    emit_late_ab(p)
                epi_q.extend(make_epilogue(w))
            for t in epi_q:
                t()

    return nc


def _split_multi_waits(nc):
    """This walrus build accepts at most one semaphore wait per instruction;
    Tile emits several (incl. its tail drain). Hoist extra waits onto
    single-wait engine NoOps inserted just before the instruction."""
    cnt = 0
    for f in nc.m.functions:
        for bb in f.blocks:
            insts = list(bb.instructions)
            out = []
            changed = False
            for inst in insts:
                si = getattr(inst, "sync_info", None)
                ws = list(si.on_wait) if (si is not None and si.on_wait) else []
                if len(ws) > 1:
                    changed = True
                    for w in ws[:-1]:
                        cnt += 1
                        out.append(mybir.InstNoOp(
                            name=f"WSPLIT-{cnt}",
                            engine=inst.engine,
                            ins=[], outs=[],
                            sync_info=mybir.SyncInfo(on_wait=[w], on_update=[]),
                        ))
                    si.on_wait = [ws[-1]]
                    inst.sync_info = si
                out.append(inst)
            if changed:
                bb.instructions = out
    return nc


def _get_program(Asz, Bsz):
    key = (Asz, Bsz)
    if key not in _PROGRAM_CACHE:
        _PROGRAM_CACHE[key] = _split_multi_waits(_build(Asz, Bsz))
    return _PROGRAM_CACHE[key]


def _prep_inputs(features_a, features_b, W_qkv, Asz, Bsz, ncores):
    """Host-side: cast to fp8, fold 80^-1/4 into W_q and W_k, reshape."""
    f8 = ml_dtypes.float8_e4m3
    fa = features_a.reshape(Asz, KC, 128, N).astype(f8)
    wt = W_qkv.T.copy().astype(np.float32)   # [640, 240]
    wt[:, 0:2 * D] *= SCALE4
    wt = wt.astype(f8).reshape(KC, 128, 240)
    fbs = []
    for c in range(ncores):
        fbs.append(features_b[c * Bsz:(c + 1) * Bsz]
                   .reshape(Bsz, KC, 128, N).astype(f8))
    return fa, fbs, wt


def kernel(features_a, features_b, W_qkv):
    Asz = features_a.shape[0]
    Bfull = features_b.shape[0]
    ncores = NCORES
    Bsz = Bfull // ncores
    fa, fbs, wt = _prep_inputs(
        np.asarray(features_a), np.asarray(features_b), np.asarray(W_qkv),
        Asz, Bsz, ncores,
    )
    nc = _get_program(Asz, Bsz)
    in_maps = [{"fa": fa, "fb": fbs[c], "wt": wt} for c in range(ncores)]
    res = run_bass_kernel_spmd(nc, in_maps, core_ids=list(range(ncores)))
    out = np.concatenate([res.results[c]["sim"].T for c in range(ncores)], axis=0)
    return out.astype(np.float32)


# revision 4
# speedup vs baseline: 1.0245x; 1.0121x over previous
"""Trainium2 Bass kernel for nn_CLARM_56693568307877 (v2, fp8 DoubleRow).

Computes, for feature sets A [64,640,14,14] and B [128,640,14,14] and a QKV
projection W [240,640]:
    q,k,v = split(x^T W^T); S = q_b k_a^T / sqrt(80); P = softmax(S)
    rec = P v_a;  sim[b,a] = -||v_b - rec||^2_F
Output [128, 64] fp32.

Sharding: data-parallel over the b batch (16 per core x 8 cores);
features_a / W replicated.

v2 design (per core: 16 b x 64 a, N=196 tokens, D=80):
  All matmuls run fp8e4 with DoubleRow perf mode (0.5 cyc/row):
    qkv:   x fp8 [128,5,196], W fp8; q/k produced as two 40-col groups so
           the [40,2,196] (d folded 2x40) DR layout for mm1 falls out of a
           plain 2-bank PSUM->SBUF copy; v_a is produced directly n-major
           (stationary = x chunk, moving = W_v) into vaug [128,2(mc),80].
    mm1:   S^T = k^T.T q^T per (a, 4b): 4 DR matmuls -> 4 PSUM banks.
    exp:   one wide ACT instruction over all 4 banks, bias -2.5 folded
           (keeps E in fp8e4 range), output E fp8 [128,2(mc),784].
    mm2:   U = vaug^T E: 2 DR matmuls (contraction 196 fits one 256-row
           DR pass) -> U [80, 784] PSUM.
  Epilogue avoids any w-broadcast / den-gather DMAs via
    sim = sum_n (2 alpha[n] w[n] - beta[n] w[n]^2) - ||v_b||^2,
    beta = sum_d U^2, alpha = sum_d U v_b, w = 1/(sum_m E):
  P2=U*U and Pv=U*(2 v_b) (DVE/Pool) are reduced over d by one-hot
  stationary PE matmuls that accumulate each pair's row into a wave-shared
  PSUM bank ([64, 392] = beta|alpha per pair); denominators s likewise via
  a ones-stationary DR matmul over E into a second wave bank. The per-wave
  tail is 6 small DVE ops + one output DMA.

Note: this walrus build accepts at most one semaphore wait per instruction
(_split_multi_waits), rejects InstTensorTensorReduce / custom DVE ops /
gpsimd-PSUM access / partition_broadcast.
"""

import numpy as np
import ml_dtypes

import concourse.bass as bass
import concourse.tile as tile
from concourse import mybir
from concourse.bass_utils import run_bass_kernel_spmd

BF16 = mybir.dt.bfloat16
F32 = mybir.dt.float32
FP8 = mybir.dt.float8e4
DR = mybir.MatmulPerfMode.DoubleRow

NCORES = 8
A_FULL = 64
B_FULL = 128
HID = 640
KC = HID // 128  # 5
N = 196          # tokens (14*14)
D = 80           # inner dim
EXP_SHIFT = 2.5  # exp(S - EXP_SHIFT); cancels in softmax, keeps E in fp8 range
SCALE4 = 1.0 / (D ** 0.25)  # folded into both W_q and W_k

_PROGRAM_CACHE = {}


def _build(Asz, Bsz):
    assert Bsz % 4 == 0 and Asz % 4 == 0
    NSG = Bsz // 4              # subgroups per a (4 b's each)
    WA = 8                      # a's per wave
    PW = WA * Bsz               # pairs per wave (128)

    nc = bass.Bass("TRN2", debug=False)
    fa = nc.dram_tensor("fa", [Asz, 128, KC, 224], FP8, kind="ExternalInput")
    fb = nc.dram_tensor("fb", [Bsz, 128, KC, 224], FP8, kind="ExternalInput")
    wt = nc.dram_tensor("wt", [KC, 128, 240], FP8, kind="ExternalInput")
    simo = nc.dram_tensor("sim", [Asz, Bsz], F32, kind="ExternalOutput")

    Exp = mybir.ActivationFunctionType.Exp
    mult = mybir.AluOpType.mult
    sub = mybir.AluOpType.subtract
    add = mybir.AluOpType.add
    X = mybir.AxisListType.X

    with tile.TileContext(nc) as tc:
        with (
            tc.tile_pool(name="const", bufs=1) as cpool,
            tc.tile_pool(name="ring", bufs=1, space="PSUM") as rpool,
            tc.tile_pool(name="e", bufs=6) as e_pool,
            tc.tile_pool(name="u", bufs=7) as u_pool,
            tc.tile_pool(name="t", bufs=7) as t_pool,
            tc.tile_pool(name="wv", bufs=2) as wv_pool,
        ):
            wt_sb = cpool.tile([128, KC, 240], FP8, tag="wt")
            kT = cpool.tile([40, 2, Asz, N], FP8, tag="kT")
            qT = cpool.tile([40, 2, Bsz, N], FP8, tag="qT")
            vaug = cpool.tile([128, 2, Asz, 97], FP8, tag="vaug")
            vbT2 = cpool.tile([D, Bsz, N], BF16, tag="vbT2")
            ohb = cpool.tile([D, 63], BF16, tag="ohb")       # one-hot cols @31
            ohs96 = cpool.tile([128, 63], BF16, tag="ohs96")  # row 96 hot @31
            ebias = cpool.tile([128, 1], F32, tag="ebias")
            vn64 = cpool.tile([PW, 1], F32, tag="vn64")
            ring = rpool.tile([128, 8, 512], F32, tag="ring")

            nc.sync.dma_start(wt_sb, wt.ap().rearrange("k p c -> p k c"))
            nc.gpsimd.memset(vaug[:], 0.0)   # mc1 rows 68:128 must stay 0
            nc.gpsimd.memset(ohb[:], 0.0)
            nc.gpsimd.memset(ohb[:, 31:32], 1.0)
            nc.gpsimd.memset(ohs96[:], 0.0)
            nc.gpsimd.memset(ohs96[96:97, 31:32], 1.0)
            # ones column at partition 96 of vaug: mm2 then emits the softmax
            # denominator s = sum_m E as row 96 of U for free (mc1 masked to
            # its 68 valid rows)
            nc.gpsimd.memset(vaug[:, 0, :, 96:97], 1.0)
            nc.gpsimd.memset(vaug[0:68, 1, :, 96:97], 1.0)
            nc.gpsimd.memset(ebias[:], -EXP_SHIFT)
            # HW PSUM powers up with undefined bits; exp reads rows 68:128 of
            # the mc1 mm1 banks (never written by the 68-row matmuls), so any
            # stale NaN there would poison mm2 via NaN*0. Zero them once.
            nc.vector.memset(ring[64:128, 0:8, :], 0.0)

            def qkv_mm(xt, c0, slot, nrows):
                """Accumulate W[:, c0:c0+nrows]^T x into ring slot.
                3 matmuls: 2 DoubleRow over kc pairs + 1 plain for kc 4."""
                out = ring[0:nrows, slot, 0:N]
                for t in range(2):
                    nc.tensor.matmul(
                        out, wt_sb[:, 2 * t:2 * t + 2, c0:c0 + nrows],
                        xt[:, 2 * t:2 * t + 2, 0:N],
                        start=(t == 0), stop=False, perf_mode=DR)
                nc.tensor.matmul(out, wt_sb[:, 4, c0:c0 + nrows],
                                 xt[:, 4, 0:N], start=False, stop=True)

            # ---- phase 1b: q, v for the 16 b batches; ||v_b||^2 ----
            # x ring: 8 slots, filled 4 batches per DMA (1120B descriptors)
            xring = cpool.tile([128, 16, KC, 224], FP8, tag="xring")

            def load4(src, i0, g=4):
                nc.sync.dma_start(
                    xring[:, (i0 % 16):(i0 % 16) + g, :, :],
                    src.ap()[i0:i0 + g].rearrange("a p k n -> p a k n"))

            for b in range(Bsz):
                if b % 2 == 0:
                    load4(fb, b, 2)
                xt = xring[:, b % 16, :, :]
                s0 = 0 if b % 2 == 0 else 4
                qkv_mm(xt, 0, s0, 40)     # q cols 0:40  -> d 0:40
                qkv_mm(xt, 40, s0 + 1, 40)  # q cols 40:80 -> d 40:80
                qkv_mm(xt, 160, s0 + 2, D)  # v (d-major, 80 rows)
                if b % 2 == 0:
                    nc.vector.tensor_copy(qT[:, :, b, :], ring[0:40, s0:s0 + 2, 0:N])
                else:
                    nc.scalar.copy(qT[:, :, b, :], ring[0:40, s0:s0 + 2, 0:N])
                nc.scalar.mul(vbT2[:, b, :], ring[0:D, s0 + 2, 0:N], 2.0)
                # ||2 v_b||^2 per token -> hot row b of slot 3
                vsq = t_pool.tile([D, N], BF16, tag="vsq")
                nc.vector.tensor_tensor(vsq, vbT2[:, b, :], vbT2[:, b, :], op=mult)
                nc.tensor.matmul(ring[0:32, 3, 0:N], ohb[:, 31 - b:63 - b], vsq,
                                 start=(b == 0), stop=(b == Bsz - 1),
                                 tile_position=(0, 0))
            vn16 = cpool.tile([Bsz, 1], F32, tag="vn16")
            nc.vector.reduce_sum(out=vn16, in_=ring[0:Bsz, 3, 0:N], axis=X)
            for g in range(PW // Bsz):
                nc.sync.dma_start(vn64[g * Bsz:(g + 1) * Bsz, :], vn16[:])

            # ---- phase 1a: k, vaug for the 64 a batches ----
            for a in range(Asz):
                if a % 2 == 0:
                    load4(fa, a, 2)
                xt = xring[:, a % 16, :, :]
                s0 = 0 if a % 2 == 0 else 4
                qkv_mm(xt, 80, s0, 40)    # k cols 80:120  -> d 0:40
                qkv_mm(xt, 120, s0 + 1, 40)  # k cols 120:160 -> d 40:80
                # v_a n-major: stationary = x chunk, moving = W_v
                for mc in range(2):
                    nrow = 128 if mc == 0 else 68
                    out = ring[0:nrow, s0 + 2 + mc, 0:D]
                    for t in range(2):
                        nc.tensor.matmul(
                            out, xt[:, 2 * t:2 * t + 2, mc * 128:mc * 128 + nrow],
                            wt_sb[:, 2 * t:2 * t + 2, 160:240],
                            start=(t == 0), stop=False, perf_mode=DR)
                    nc.tensor.matmul(out, xt[:, 4, mc * 128:mc * 128 + nrow],
                                     wt_sb[:, 4, 160:240], start=False, stop=True)
                if a % 2 == 0:
                    nc.vector.tensor_copy(kT[:, :, a, :], ring[0:40, s0:s0 + 2, 0:N])
                    nc.scalar.copy(vaug[0:128, 0, a, 0:D], ring[0:128, s0 + 2, 0:D])
                    nc.scalar.copy(vaug[0:68, 1, a, 0:D], ring[0:68, s0 + 3, 0:D])
                else:
                    nc.scalar.copy(kT[:, :, a, :], ring[0:40, s0:s0 + 2, 0:N])
                    nc.vector.tensor_copy(vaug[0:128, 0, a, 0:D], ring[0:128, s0 + 2, 0:D])
                    nc.vector.tensor_copy(vaug[0:68, 1, a, 0:D], ring[0:68, s0 + 3, 0:D])

            # ---- phase 2: 16 waves x (4 a x 4 subgroups) ----
            # 2-deep software pipeline per wave:
            #   sg t emits: mm1(t), exp(t) | mm2/s/egress/P2/Pv of t-1 |
            #   alpha/beta matmuls of t-2 (so they never head-block mm1).

            def emit_early(p):
                es, a, jbase = p
                # mm2: U = vaug^T E  [80, 784] into slots 4,5
                with tc.high_priority(offset=500000):
                    for ncx in range(2):
                        nc.tensor.matmul(
                            ring[0:97, 4 + ncx, 0:392], vaug[:, :, a, :],
                            es[:, :, ncx * 392:(ncx + 1) * 392],
                            start=True, stop=True, perf_mode=DR)
                # U egress (row 96 = denominators) + P2/Pv products, in
                # ncx halves so the Pool Pv can start ~1.4us earlier (the
                # egress->Pv leg sits on the pipeline's pacing loop)
                u_sb = u_pool.tile([97, 2, 392], BF16, tag="u")
                tt = t_pool.tile([D, 4, 2, N], BF16, tag="t")
                b0_ = jbase % Bsz
                for h in range(2):
                    nc.vector.tensor_copy(
                        u_sb[:, h, :], ring[0:97, 4 + h, 0:392])
                    uvh = u_sb[0:D, h, :].rearrange("p (j n) -> p j n", j=2)
                    nc.gpsimd.tensor_tensor(
                        tt[:, 2 * h:2 * h + 2, 1, :], uvh,
                        vbT2[:, b0_ + 2 * h:b0_ + 2 * h + 2, :], op=mult)
                    nc.vector.tensor_tensor(
                        tt[:, 2 * h:2 * h + 2, 0, :], uvh, uvh, op=mult)
                return tt, u_sb, jbase

            def emit_late_s(p):
                # denominators s -> hot rows of slot 7 (1-partition
                # stationary reading u_sb row 96)
                tt, u_sb, jbase = p
                for p_ in range(4):
                    j = jbase + p_
                    blk, off = divmod(j, 32)
                    nc.tensor.matmul(
                        ring[32 * blk:32 * blk + 32, 7, 0:N],
                        ohs96[96:97, 31 - off:63 - off],
                        u_sb[96:97, p_ // 2, (p_ % 2) * N:(p_ % 2) * N + N],
                        start=(off == 0), stop=(off == 31),
                        tile_position=(96, 32 * blk))

            def emit_late_ab(p):
                # beta|alpha -> hot rows of slot 6
                tt, u_sb, jbase = p
                for p_ in range(4):
                    j = jbase + p_
                    blk, off = divmod(j, 32)
                    nc.tensor.matmul(
                        ring[32 * blk:32 * blk + 32, 6, 0:392],
                        ohb[:, 31 - off:63 - off], tt[:, p_, :, :],
                        start=(off == 0), stop=(off == 31),
                        tile_position=(0, 32 * blk))

            epi_q = []

            def make_epilogue(w):
                """Per-wave tail as individually emittable thunks so the DVE
                work spreads across the next wave instead of bursting."""
                wb = wv_pool.tile([PW, N], BF16, tag="wb")
                tb = wv_pool.tile([PW, N], BF16, tag="tb")
                z1 = wv_pool.tile([PW, N], BF16, tag="z1")
                z = wv_pool.tile([PW, N], BF16, tag="z")
                rsum = wv_pool.tile([PW, 1], F32, tag="rs")
                sim64 = wv_pool.tile([PW, 1], F32, tag="sim")

                def t0():
                    with nc.allow_low_precision(reason="w in bf16 is plenty"):
                        nc.vector.reciprocal(wb, ring[0:PW, 7, 0:N])
                def t1():
                    nc.vector.tensor_tensor(tb, ring[0:PW, 6, 0:N], wb, op=mult)
                def t2():
                    nc.vector.tensor_tensor(
                        z1, ring[0:PW, 6, N:2 * N], tb, op=sub)
                def t3():
                    nc.vector.tensor_tensor(z, z1, wb, op=mult)
                def t4():
                    nc.vector.reduce_sum(out=rsum, in_=z, axis=X)
                    nc.vector.scalar_tensor_tensor(
                        out=sim64, in0=vn64, scalar=-0.25, in1=rsum,
                        op0=mult, op1=add)
                    nc.sync.dma_start(simo[WA * w:WA * (w + 1), :], sim64[:])
                return [t0, t1, t2, t3, t4]

            for w in range(Asz // WA):
                early_q = None
                late_q = []
                for a_loc in range(WA):
                    a = WA * w + a_loc
                    for sgb in range(NSG):
                        b0 = 4 * sgb
                        jbase = Bsz * a_loc + b0
                        es = e_pool.tile([128, 2, 784], FP8, tag="e")
                        # mm1 mc0 -> slots 0,1; exp_a is gated only by these
                        # two matmuls, so keep everything else off the PE
                        # queue until they are emitted; slip the old s / ab
                        # one-hot matmuls between the two exp halves.
                        pop = late_q.pop(0) if len(late_q) == 6 else None
                        # mm1+exp get top scheduler priority so the list
                        # scheduler never parks late-stage backlog in front
                        # of the exp-critical matmuls.
                        with tc.high_priority(offset=1000000):
                            for mc in range(2):
                                nrow = 128 if mc == 0 else 68
                                for ncx in range(2):
                                    nc.tensor.matmul(
                                        ring[0:nrow, 2 * mc + ncx, 0:392],
                                        kT[:, :, a, mc * 128:mc * 128 + nrow],
                                        qT[:, :, b0 + 2 * ncx:b0 + 2 * ncx + 2, :],
                                        start=True, stop=True, perf_mode=DR)
                                nc.scalar.activation(
                                    es[:, mc, :].rearrange(
                                        "p (s y) -> p s y", s=2),
                                    ring[:, 2 * mc:2 * mc + 2, 0:392],
                                    Exp, bias=ebias[:])
                        if pop is not None:
                            emit_late_s(pop)
                        if early_q is not None:
                            late_q.append(emit_early(early_q))
                        if pop is not None:
                            emit_late_ab(pop)
                        if epi_q:
                            epi_q.pop(0)()
                        early_q = (es, a, jbase)
                late_q.append(emit_early(early_q))
                for p in late_q:
                    emit_late_s(p)
                    emit_late_ab(p)
                epi_q.extend(make_epilogue(w))
            for t in epi_q:
                t()

    return nc


def _split_multi_waits(nc):
    """This walrus build accepts at most one semaphore wait per instruction;
    Tile emits several (incl. its tail drain). Hoist extra waits onto
    single-wait engine NoOps inserted just before the instruction."""
    cnt = 0
    for f in nc.m.functions:
        for bb in f.blocks:
            insts = list(bb.instructions)
            out = []
            changed = False
            for inst in insts:
                si = getattr(inst, "sync_info", None)
                ws = list(si.on_wait) if (si is not None and si.on_wait) else []
                if len(ws) > 1:
                    changed = True
                    for w in ws[:-1]:
                        cnt += 1
                        out.append(mybir.InstNoOp(
                            name=f"WSPLIT-{cnt}",
                            engine=inst.engine,
                            ins=[], outs=[],
                            sync_info=mybir.SyncInfo(on_wait=[w], on_update=[]),
                        ))
                    si.on_wait = [ws[-1]]
                    inst.sync_info = si
                out.append(inst)
            if changed:
                bb.instructions = out
    return nc


def _get_program(Asz, Bsz):
    key = (Asz, Bsz)
    if key not in _PROGRAM_CACHE:
        _PROGRAM_CACHE[key] = _split_multi_waits(_build(Asz, Bsz))
    return _PROGRAM_CACHE[key]


def _prep_inputs(features_a, features_b, W_qkv, Asz, Bsz, ncores):
    """Host-side: cast to fp8, fold 80^-1/4 into W_q and W_k, reshape."""
    f8 = ml_dtypes.float8_e4m3

    def prep(feat, n):
        x = feat.reshape(n, KC, 128, N).transpose(0, 2, 1, 3)  # [n,128,KC,N]
        out = np.zeros((n, 128, KC, 224), dtype=f8)
        out[:, :, :, 0:N] = x.astype(f8)
        return out

    fa = prep(features_a, Asz)
    wt = W_qkv.T.copy().astype(np.float32)   # [640, 240]
    wt[:, 0:2 * D] *= SCALE4
    wt = wt.astype(f8).reshape(KC, 128, 240)
    fbs = [prep(features_b[c * Bsz:(c + 1) * Bsz], Bsz) for c in range(ncores)]
    return fa, fbs, wt


def kernel(features_a, features_b, W_qkv):
    Asz = features_a.shape[0]
    Bfull = features_b.shape[0]
    ncores = NCORES
    Bsz = Bfull // ncores
    fa, fbs, wt = _prep_inputs(
        np.asarray(features_a), np.asarray(features_b), np.asarray(W_qkv),
        Asz, Bsz, ncores,
    )
    nc = _get_program(Asz, Bsz)
    in_maps = [{"fa": fa, "fb": fbs[c], "wt": wt} for c in range(ncores)]
    res = run_bass_kernel_spmd(nc, in_maps, core_ids=list(range(ncores)))
    out = np.concatenate([res.results[c]["sim"].T for c in range(ncores)], axis=0)
    return out.astype(np.float32)
